# revision 8
# baseline (speedup 1.0000x reference)
"""Sliding-window (tau=32) multi-head attention block with shared qkv projection,
distributed over 8 trn2 NeuronCores.

Sharding: data/sequence-parallel over the flattened (batch, token) axis —
8 shards of 1024 tokens. Each core receives its k/v slice with a 32-row
front halo (zeros at batch start), so projecting the concatenated buffer
reproduces the reference's pad-then-project semantics exactly (incl. bias).

Perf structure (v2):
- batched DMAs: one load per raw tensor (+tails), one const blob, one store.
- transposes + projections in f32r (1 cyc/row at free>=256 / 1.5 transpose).
- scores, exp, mask, out' matmuls in bf16 (1 cyc/row at any free size,
  vs f32r's 4 cyc/row below 256) — the small windowed matmuls dominate PE.
"""

import numpy as np
import ml_dtypes

import concourse.bacc as bacc
import concourse.bass as bass
import concourse.tile as tile
from concourse import mybir
from concourse.bass_utils import run_bass_kernel_spmd

B, N, E = 2, 4096, 256
H, TAU = 8, 32
HD = E // H
SCALING = HD**-0.5

NCORES = 8
T = B * N // NCORES  # 1024 q tokens per core
KT = T + 32  # kv rows incl. 32-row front halo
NB = KT // 32  # 33 kv blocks of 32
NQT = T // 128  # 8 q tiles of 128
NKTILE = (NB + 3) // 4  # 9 kpos tiles of (up to) 4 blocks

F32 = mybir.dt.float32
F32R = mybir.dt.float32r
BF16 = mybir.dt.bfloat16

# const blob column offsets (f32 units)
_IDENT0 = 0
_WT0 = 128
_WTAUG0 = _WT0 + 512
_MASK0 = _WTAUG0 + 2 * (H * 33)          # masks stored bf16-packed-in-f32
_B20 = _MASK0 + 3 * (H * 64) // 2
_B2S0 = _B20 + 2
_ONES0 = _B2S0 + 2
_BAUG0 = _ONES0 + 128
_CBLOB = _BAUG0 + H * 33


def _host_constants():
    """Band masks in S^T window coords: rows jj (kpos within block), cols ii
    (q within the 64-wide window); valid iff ii - jj in [0, 31]."""
    jj = np.arange(32)[:, None]
    ii = np.arange(64)[None, :]
    band = ((ii - jj >= 0) & (ii - jj <= 31)).astype(np.float32)  # [32, 64]
    band128 = np.tile(band, (4, 1))  # [128, 64]
    normal = np.repeat(band128[:, None, :], H, axis=1)  # [128, H, 64]
    first = normal.copy()
    # kpos tile 0, block m=0 (partitions 0:32): left window half (q block -1)
    # does not exist.
    first[0:32, :, 0:32] = 0.0
    last = np.zeros_like(normal)
    # kpos tile 8 holds only block m=32 (partitions 0:32); only its left
    # window half (q block 31) exists.
    last[0:32, :, 0:32] = np.repeat(band[:, 0:32][:, None, :], H, axis=1)
    masks = np.stack([normal.reshape(128, H * 64),
                      first.reshape(128, H * 64),
                      last.reshape(128, H * 64)])  # [3, 128, 512]
    return masks.astype(np.float32)


def _make_const_blob(W, b):
    WT = np.ascontiguousarray(W.T).astype(np.float32)  # [e_in, e_out]
    wT = WT.reshape(2, 128, 256)
    # augmented V weights: per-head 33-wide column groups, ones col slot = 0
    WTaug = np.zeros((256, H * 33), np.float32)
    for h in range(H):
        WTaug[:, 33 * h : 33 * h + 32] = WT[:, 32 * h : 32 * h + 32]
    wTaug = WTaug.reshape(2, 128, H * 33)
    b_aug = np.zeros((H * 33,), np.float32)
    for h in range(H):
        b_aug[33 * h : 33 * h + 32] = b[32 * h : 32 * h + 32]
        b_aug[33 * h + 32] = 1.0
    b2 = b.reshape(2, 128).astype(np.float32)

    masks = _host_constants().astype(ml_dtypes.bfloat16)  # [3, 128, 512]
    masks_f32 = masks.view(np.uint16).reshape(3, 128, 256, 2)
    masks_f32 = np.ascontiguousarray(masks_f32).view(np.float32)[..., 0]

    blob = np.zeros((128, _CBLOB), np.float32)
    blob[:, _IDENT0:_IDENT0 + 128] = np.eye(128, dtype=np.float32)
    blob[:, _WT0:_WT0 + 512] = wT.transpose(1, 0, 2).reshape(128, 512)
    blob[:, _WTAUG0:_WTAUG0 + 528] = wTaug.transpose(1, 0, 2).reshape(128, 528)
    # pack bf16 masks: pairs of bf16 -> one f32 lane, layout [128, 3*256]
    mu16 = np.ascontiguousarray(masks.view(np.uint16))          # [3,128,512]
    mpk = mu16.reshape(3, 128, 256, 2)
    mf32 = np.ascontiguousarray(mpk).view(np.uint32)[..., 0]    # [3,128,256]
    blob[:, _MASK0:_MASK0 + 768] = mf32.transpose(1, 0, 2).reshape(
        128, 768).view(np.float32)
    blob[:, _B20:_B20 + 2] = b2.T
    blob[:, _B2S0:_B2S0 + 2] = (SCALING * b2).T
    blob[0, _ONES0:_ONES0 + 128] = 1.0
    blob[0, _BAUG0:_BAUG0 + H * 33] = b_aug
    return blob


def build_program(stage=4, reps=1):
    _ = stage
    nc = bacc.Bacc("TRN2", target_bir_lowering=False)

    q_d = nc.dram_tensor("q", [T, E], F32, kind="ExternalInput")
    k_d = nc.dram_tensor("k", [KT, E], F32, kind="ExternalInput")
    v_d = nc.dram_tensor("v", [KT, E], F32, kind="ExternalInput")
    cb_d = nc.dram_tensor("cblob", [128, _CBLOB], F32, kind="ExternalInput")
    out_d = nc.dram_tensor("out", [T, E], F32, kind="ExternalOutput")

    with tile.TileContext(nc) as tc:
        with (
            tc.tile_pool(name="consts", bufs=1) as consts,
            tc.tile_pool(name="raw", bufs=1) as raw_pool,
            tc.tile_pool(name="xT", bufs=1) as xT_pool,
            tc.tile_pool(name="proj", bufs=1) as proj_pool,
            tc.tile_pool(name="aw", bufs=1) as aw_pool,
            tc.tile_pool(name="ofin", bufs=4) as ofin_pool,
            tc.tile_pool(name="oall", bufs=1) as oall_pool,
            tc.tile_pool(name="ps_proj", bufs=2, space="PSUM") as ps_proj,
            tc.tile_pool(name="ps_s", bufs=1, space="PSUM") as ps_s,
            tc.tile_pool(name="ps_o", bufs=1, space="PSUM") as ps_o,
        ):
            # ---- constants: one DMA + bitcast views -----------------------
            blob = consts.tile([128, _CBLOB], F32)
            nc.sync.dma_start(out=blob, in_=cb_d.ap())
            ident = blob[:, _IDENT0:_IDENT0 + 128]
            masks_sb = blob[:, _MASK0:_MASK0 + 768].bitcast(BF16).rearrange(
                "p (i w) -> p i w", i=3)  # [128, 3, 512] bf16
            b2_sb = blob[:, _B20:_B20 + 2]
            b2s_sb = blob[:, _B2S0:_B2S0 + 2]
            # f32r is a distinct stored format: convert via rounding copies
            wT_fr = consts.tile([128, 2, 256], F32R)
            nc.vector.tensor_copy(
                wT_fr, blob[:, _WT0:_WT0 + 512].rearrange(
                    "p (k e) -> p k e", k=2))
            wTaug_fr = consts.tile([128, 2, H * 33], F32R)
            nc.vector.tensor_copy(
                wTaug_fr, blob[:, _WTAUG0:_WTAUG0 + 528].rearrange(
                    "p (k e) -> p k e", k=2))
            ones_fr = consts.tile([1, 128], F32R)
            nc.vector.tensor_copy(ones_fr, blob[0:1, _ONES0:_ONES0 + 128])
            baug_fr = consts.tile([1, H * 33], F32R)
            nc.vector.tensor_copy(baug_fr, blob[0:1, _BAUG0:_BAUG0 + H * 33])

            for _rep in range(reps):
              _ = _rep  # noqa
              # ---- batched raw loads --------------------------------------
              rawq = raw_pool.tile([128, NQT, E], F32, tag="rawq")
              rawk = raw_pool.tile([128, 9, E], F32, tag="rawk")
              rawv = raw_pool.tile([128, 9, E], F32, tag="rawv")
              nc.sync.dma_start(
                  out=rawq, in_=q_d.ap().rearrange("(c p) e -> p c e", p=128))
              nc.sync.dma_start(
                  out=rawk[:, 0:8, :],
                  in_=k_d.ap()[0:1024].rearrange("(c p) e -> p c e", p=128))
              nc.sync.dma_start(out=rawk[0:32, 8, :], in_=k_d.ap()[1024:KT])
              nc.sync.dma_start(
                  out=rawv[:, 0:8, :],
                  in_=v_d.ap()[0:1024].rearrange("(c p) e -> p c e", p=128))
              nc.sync.dma_start(out=rawv[0:32, 8, :], in_=v_d.ap()[1024:KT])

              # ---- PE transpose -> xT (f32r) ------------------------------
              xT_q = xT_pool.tile([128, 2, T], F32R, tag="xTq")
              xT_k = xT_pool.tile([128, 2, KT], F32R, tag="xTk")
              xT_v = xT_pool.tile([128, 2, KT], F32R, tag="xTv")

              def load_transpose(raw, xT, nchunk):
                  chunks = [(c, 128) for c in range(min(nchunk, 8))]
                  if nchunk == 9:
                      chunks.append((8, 32))
                  pairs = [chunks[i:i + 2] for i in range(0, len(chunks), 2)]
                  for idx, pair in enumerate(pairs):
                      pt = ps_proj.tile([128, 512], F32, tag="psp",
                                        name="pt").rearrange(
                          "p (a b) -> p a b", a=2)
                      base = pair[0][0] * 128
                      tot = sum(pc for _, pc in pair)
                      for j, (c, pc) in enumerate(pair):
                          rt = raw[:, c, :]
                          for o in range(2):
                              nc.tensor.transpose(
                                  pt[:, o, 128 * j : 128 * j + pc],
                                  rt[:pc, 128 * o : 128 * o + 128],
                                  ident[:pc, :pc],
                              )
                      if idx % 2 == 0:
                          nc.scalar.activation(
                              xT[:, :, base : base + tot],
                              pt[:, :, :tot],
                              mybir.ActivationFunctionType.Copy,
                          )
                      else:
                          nc.vector.tensor_copy(
                              xT[:, :, base : base + tot], pt[:, :, :tot]
                          )

              load_transpose(rawq, xT_q, NQT)
              load_transpose(rawk, xT_k, 9)
              load_transpose(rawv, xT_v, 9)

              # ---- q/k projections -> bf16 transposed layout --------------
              qpT = proj_pool.tile([128, 2, T], BF16, tag="qpT")
              kpT = proj_pool.tile([128, 2, KT], BF16, tag="kpT")

              def project_T(xT, outT, tok_total, bias_sb, scale):
                  j = 0
                  drain_idx = 0
                  while j < tok_total:
                      w = min(512, tok_total - j)
                      for o in range(2):
                          ps = ps_proj.tile([128, 512], F32, tag="psp")
                          for ki in range(2):
                              nc.tensor.matmul(
                                  ps[:, :w],
                                  wT_fr[:, ki, 128 * o : 128 * o + 128],
                                  xT[:, ki, j : j + w],
                                  start=(ki == 0),
                                  stop=(ki == 1),
                              )
                          if drain_idx % 2 == 0:
                              nc.scalar.activation(
                                  outT[:, o, j : j + w],
                                  ps[:, :w],
                                  mybir.ActivationFunctionType.Identity,
                                  bias=bias_sb[:, o : o + 1],
                                  scale=scale,
                              )
                          else:
                              nc.vector.tensor_scalar(
                                  outT[:, o, j : j + w],
                                  ps[:, :w],
                                  scale,
                                  bias_sb[:, o : o + 1],
                                  mybir.AluOpType.mult,
                                  mybir.AluOpType.add,
                              )
                          drain_idx += 1
                      j += w

              project_T(xT_q, qpT, T, b2_sb, 1.0)
              project_T(xT_k, kpT, KT, b2s_sb, SCALING)

              # ---- v_aug projection (natural layout, bf16) ----------------
              kv_chunks = [(c * 128, 128) for c in range(8)] + [(1024, 32)]
              vpa = [
                  proj_pool.tile([128, H * 33], BF16, tag=f"vpa{i}",
                                 name=f"vpa{i}")
                  for i in range(9)
              ]
              for idx, (c0, pc) in enumerate(kv_chunks):
                  ps = ps_proj.tile([128, 512], F32, tag="psp")
                  for ki in range(2):
                      nc.tensor.matmul(
                          ps[:pc, 0 : H * 33],
                          xT_v[:, ki, c0 : c0 + pc],
                          wTaug_fr[:, ki, :],
                          start=(ki == 0),
                          stop=False,
                      )
                  nc.tensor.matmul(
                      ps[:pc, 0 : H * 33],
                      ones_fr[:, :pc],
                      baug_fr,
                      start=False,
                      stop=True,
                  )
                  if idx % 2 == 0:
                      nc.vector.tensor_copy(vpa[idx][:pc, :], ps[:pc, 0 : H * 33])
                  else:
                      nc.scalar.activation(
                          vpa[idx][:pc, :], ps[:pc, 0 : H * 33],
                          mybir.ActivationFunctionType.Copy,
                      )

              # ---- scores (S^T windowed, bf16) + exp + mask ---------------
              # PSUM layout: [128 (sig,jj), 4 (hr -> bank), 128 (ht,64win)].
              aw = [
                  aw_pool.tile([128, 4, 128], BF16, tag=f"aw{c}",
                               name=f"aw{c}")
                  for c in range(NKTILE)
              ]
              ofin_all = oall_pool.tile([128, NQT, H, 32], F32, tag="oall")

              # ---- out' matmuls + normalize -------------------------------
              def out_tile(t):
                  po = ps_o.tile([128, 2, H, 64], F32, tag="pso", name="po")
                  for gi in range(4):
                      g = 4 * t + gi
                      for h in range(H):
                          hr, ht = h % 4, h // 4
                          for mi, m in enumerate((g, g + 1)):
                              c, sig = m // 4, m % 4
                              half = 32 if m == g else 0
                              lhsT = aw[c][
                                  32 * sig : 32 * sig + 32, hr,
                                  64 * ht + half : 64 * ht + half + 32,
                              ]
                              rhs = vpa[c][
                                  32 * sig : 32 * sig + 32, 33 * h : 33 * h + 33
                              ]
                              nc.tensor.matmul(
                                  po[32 * gi : 32 * gi + 32, mi, h, 0:33],
                                  lhsT,
                                  rhs,
                                  start=True,
                                  stop=True,
                                  tile_position=(32 * sig, 32 * gi),
                              )
                  pb_sb = ofin_pool.tile([128, H, 33], F32, tag="pb_sb")
                  nc.scalar.activation(
                      pb_sb, po[:, 1, :, 0:33], mybir.ActivationFunctionType.Copy
                  )
                  osum = ofin_pool.tile([128, H, 33], F32, tag="osum")
                  nc.vector.scalar_tensor_tensor(
                      out=osum,
                      in0=po[:, 0, :, 0:33],
                      scalar=1.0,
                      in1=pb_sb,
                      op0=mybir.AluOpType.mult,
                      op1=mybir.AluOpType.add,
                  )
                  rec = ofin_pool.tile([128, H], F32, tag="rec")
                  nc.vector.reciprocal(rec, osum[:, :, 32])
                  rec_b = bass.AP(
                      tensor=rec.tensor,
                      offset=rec.offset,
                      ap=[rec.ap[0], [rec.ap[1][0], H], [0, 32]],
                  )
                  nc.vector.tensor_mul(
                      ofin_all[:, t], osum[:, :, 0:32], rec_b)

              for c in range(NKTILE):
                  nsig = 4 if c < NKTILE - 1 else NB - 4 * c
                  ps = ps_s.tile([128, 4, 128], F32, tag="pss",
                                 padded_shape=[128, 4, 512])
                  if c == NKTILE - 1:
                      nc.vector.memset(ps[:, :, 0:128], 0.0)
                  for sig in range(nsig):
                      m = 4 * c + sig
                      for h in range(H):
                          hr, ht = h % 4, h // 4
                          lhsT = kpT[32 * hr : 32 * hr + 32, ht,
                                     32 * m : 32 * m + 32]
                          if m == 0:
                              rhs = qpT[32 * hr : 32 * hr + 32, ht, 0:32]
                              outap = ps[32 * sig : 32 * sig + 32, hr,
                                         64 * ht + 32 : 64 * ht + 64]
                          elif m == NB - 1:
                              rhs = qpT[
                                  32 * hr : 32 * hr + 32, ht,
                                  32 * (m - 1) : 32 * m
                              ]
                              outap = ps[32 * sig : 32 * sig + 32, hr,
                                         64 * ht : 64 * ht + 32]
                          else:
                              rhs = qpT[
                                  32 * hr : 32 * hr + 32, ht,
                                  32 * (m - 1) : 32 * (m + 1),
                              ]
                              outap = ps[32 * sig : 32 * sig + 32, hr,
                                         64 * ht : 64 * ht + 64]
                          nc.tensor.matmul(
                              outap,
                              lhsT,
                              rhs,
                              start=True,
                              stop=True,
                              tile_position=(32 * hr, 32 * sig),
                          )
                  # zero never-written PSUM regions so exp sees finite values
                  if c == 0:
                      nc.vector.memset(ps[0:32, :, 0:32], 0.0)
                      nc.vector.memset(ps[0:32, :, 64:96], 0.0)
                  ex = aw_pool.tile([128, 4, 128], BF16, tag="ex", bufs=3)
                  nc.scalar.activation(ex, ps[:, :, 0:128],
                                       mybir.ActivationFunctionType.Exp)
                  mi = 0 if 0 < c < NKTILE - 1 else (1 if c == 0 else 2)
                  nc.vector.tensor_mul(aw[c], ex, masks_sb[:, mi, :].rearrange(
                      "p (r w) -> p r w", r=4))
                  if c >= 1:
                      out_tile(c - 1)

              # ---- single batched store -----------------------------------
              nc.sync.dma_start(
                  out=out_d.ap().rearrange("(t p) e -> p t e", p=128),
                  in_=ofin_all.rearrange("p t h w -> p t (h w)"),
              )

    nc.compile()
    return nc


_NC_CACHE = None


def _get_nc():
    global _NC_CACHE
    if _NC_CACHE is None:
        _NC_CACHE = build_program()
    return _NC_CACHE


def make_in_maps(query, key, value, W, b):
    query = np.asarray(query, np.float32)
    key = np.asarray(key, np.float32)
    value = np.asarray(value, np.float32)
    W = np.asarray(W, np.float32)
    b = np.asarray(b, np.float32)

    cblob = _make_const_blob(W, b)

    qf = query.reshape(B * N, E)
    kf = key.reshape(B * N, E)
    vf = value.reshape(B * N, E)
    shards_per_b = NCORES // B
    in_maps = []
    for c in range(NCORES):
        s0 = c * T
        halo0 = s0 - 32
        if c % shards_per_b == 0:
            halo_k = np.zeros((32, E), np.float32)
            halo_v = np.zeros((32, E), np.float32)
        else:
            halo_k = kf[halo0:s0]
            halo_v = vf[halo0:s0]
        in_maps.append(
            {
                "q": np.ascontiguousarray(qf[s0 : s0 + T]),
                "k": np.ascontiguousarray(np.concatenate([halo_k, kf[s0 : s0 + T]])),
                "v": np.ascontiguousarray(np.concatenate([halo_v, vf[s0 : s0 + T]])),
                "cblob": cblob,
            }
        )
    return in_maps


def kernel(query, key, value, W, b):
    nc = _get_nc()
    in_maps = make_in_maps(query, key, value, W, b)
    res = run_bass_kernel_spmd(nc, in_maps, list(range(NCORES)))
    out = np.concatenate([res.results[c]["out"] for c in range(NCORES)], axis=0)
    return out.reshape(B, N, E).astype(np.float32)
